# revision 5
# baseline (speedup 1.0000x reference)
"""SimpleGCN (3-layer GCNConv + global_add_pool + linear head) on 8 Trainium2 cores.

v2 strategy (shapes hardcoded for nn_SimpleGCN):
 - Nodes sharded contiguously across 8 cores by dst (12500 each).
 - Broadcast value per layer l: b_l = (h_{l-1} @ W_l) * dinv  (bf16, 128 cols,
   no duplication). b_0 computed in a startup loop from x; b_{l+1} fused into
   layer l's per-block epilogue (transpose -> W matmul -> dinv scale -> DMA
   into agin slice), so no separate phase-A pass and no hT DRAM round trip.
 - agin is split into 4 block-aligned slices (25/25/25/23 blocks); each slice
   gets its own AllGather fired as soon as its last block is written, so the
   collectives overlap the tail of phase B and the next layer's gathers.
 - Gather source agout_q = [8 ranks x szq rows, 128] bf16; edges bucketed by
   (dst-block, src-slice-quarter); idx = rank*szq + local offset (< 25600,
   int16-safe). dma_gather pulls 256B rows in windows of 28 chunks (3584 idx)
   on 4 SWDGE queues.
 - One-hot S built on-chip (is_equal of dst-local vs iota, bf16); PE matmul
   accumulates per-128-dst-block segment sums in PSUM.
 - Epilogue: h = relu(dinv*psum + bias); layer 2 pools via matmul into a
   per-core local-graph window; head matmul gives partial logits; host sums.
"""
import math
import numpy as np

N_NODES = 100000
N_EDGES = 1600000
D = 128
L = 3
G = 512
NC = 8
SH = N_NODES // NC            # 12500 nodes per core
NBLK = math.ceil(SH / 128)    # 98 blocks (97 full + one of 84)
BW = [128] * (NBLK - 1) + [SH - 128 * (NBLK - 1)]
NQ = 4
QBLK = [25, 25, 25, 23]                      # blocks per slice-quarter
QSTART = [0, 25, 50, 75]                     # first block of each quarter
QROWS = [3200, 3200, 3200, SH - 9600]        # rows per quarter (last: 2900)
QROWBASE = [0, 3200, 6400, 9600]
WCH = 28                      # chunks per gather window (3584 idx/instruction)
NIDX = WCH * 128
MSG_BUFS = 2
S_BUFS = 2
POOLW = 256                   # per-core local pooled window

BLK_Q = [min(b // 25, 3) for b in range(NBLK)]
AG_FIRE_BLOCKS = {QSTART[q] + QBLK[q] - 1: q for q in range(NQ)}  # {24:0,49:1,74:2,97:3}


def _prep(x, edge_index, batch, Ws, bs, head_w, head_b):
    x = np.asarray(x, np.float32)
    ei = np.asarray(edge_index, np.int64)
    batch = np.asarray(batch, np.int64)
    Ws = np.asarray(Ws, np.float32)
    bs = np.asarray(bs, np.float32)
    head_w = np.asarray(head_w, np.float32)

    loops = np.arange(N_NODES, dtype=np.int64)
    src = np.concatenate([ei[0], loops])
    dst = np.concatenate([ei[1], loops])
    deg = np.bincount(dst, minlength=N_NODES).astype(np.float32)
    dinv = np.where(deg > 0, 1.0 / np.sqrt(deg), 0.0).astype(np.float32)

    # src -> (slice-quarter, idx within agout_q)
    s_rank = src // SH
    s_off = src % SH
    s_q = np.minimum(s_off // 3200, 3)
    s_idx = s_rank * np.asarray(QROWS, np.int64)[s_q] + (s_off - np.asarray(QROWBASE, np.int64)[s_q])

    # ---- per-core edge bucketing by (dst block, src slice-quarter) ----
    core = dst // SH
    per_core = []
    counts = np.zeros((NC, NBLK * NQ), np.int64)
    for c in range(NC):
        m = core == c
        si_c = s_idx[m]
        sq_c = s_q[m]
        dloc = dst[m] - c * SH
        b = dloc >> 7
        key = b * NQ + sq_c
        order = np.argsort(key, kind="stable")
        counts[c] = np.bincount(key, minlength=NBLK * NQ)
        per_core.append((si_c[order], dloc[order], np.cumsum(counts[c]) - counts[c]))

    cmax = counts.max(axis=0).reshape(NBLK, NQ)            # max edges per (b,q)
    cchunks = -(-cmax // 128)                              # chunks per (b,q)
    qck_base = np.zeros((NBLK, NQ), np.int64)              # chunk offset within quarter
    CQ = np.zeros(NQ, np.int64)
    for qq in range(NQ):
        run = 0
        for b in range(NBLK):
            qck_base[b, qq] = run
            run += cchunks[b, qq]
        CQ[qq] = run
    NW = [int(-(-CQ[qq] // WCH)) for qq in range(NQ)]      # windows per quarter
    qwin_base = np.concatenate([[0], np.cumsum(NW)]).astype(np.int64)
    CTOT = int(sum(NW)) * WCH                              # total chunk slots
    NWmax = max(NW)

    # block -> list of (q, w, s) chunk refs; block ready window-group
    blk_chunks = []
    blk_ready = []
    for b in range(NBLK):
        refs = []
        wmax = 0
        for qq in range(NQ):
            for k in range(int(cchunks[b, qq])):
                ch = int(qck_base[b, qq]) + k
                refs.append((qq, ch // WCH, ch % WCH))
                wmax = max(wmax, ch // WCH)
        blk_chunks.append(refs)
        blk_ready.append(wmax)

    def quarter_chunk_col(qq, ch):   # global chunk slot column for (quarter, chunk)
        return (int(qwin_base[qq]) * WCH) + ch

    idx_cols = CTOT * (NIDX // 16) // WCH  # = CTOT * 8
    ins_per_core = []
    pooled_base = np.zeros(NC, np.int64)
    ws_blk = [max(0, int(b * 128 * G / N_NODES) - 32) for b in range(NBLK)]
    for c in range(NC):
        si_c, dloc, starts = per_core[c]
        ixf = np.zeros(CTOT * 128, np.int64)               # gather idx per slot (pad 0)
        dlf = np.full(CTOT * 128, -1.0, np.float32)        # dst-local per slot (pad -1)
        for b in range(NBLK):
            for qq in range(NQ):
                n = int(counts[c][b * NQ + qq])
                if n == 0 and cchunks[b, qq] == 0:
                    continue
                st = int(starts[b * NQ + qq])
                base = quarter_chunk_col(qq, int(qck_base[b, qq])) * 128
                ixf[base:base + n] = si_c[st:st + n]
                dlf[base:base + n] = (dloc[st:st + n] % 128).astype(np.float32)
        # wrap indices: slot j of each window -> idx[p, wcol + j//16] with p%16 == j%16
        ix_win = ixf.reshape(CTOT // WCH, NIDX)            # per window
        arr = ix_win.reshape(-1, NIDX // 16, 16)           # [win, NIDX//16, 16]
        idx_sb = np.transpose(arr, (0, 2, 1)).reshape(CTOT // WCH, 16, NIDX // 16)
        idx_sb = np.concatenate([idx_sb] * 8, axis=1)      # replicate to 128 partitions
        idx_sb = np.transpose(idx_sb, (1, 0, 2)).reshape(128, idx_cols)
        dl_sb = dlf.reshape(CTOT, 128).T.copy()            # [128, CTOT]

        xT = np.zeros((128, NBLK * 128), np.float32)
        xT[:, :SH] = x[c * SH:(c + 1) * SH].T
        dinv_c = np.ones((128, NBLK), np.float32)
        dv = dinv[c * SH:(c + 1) * SH]
        for b in range(NBLK):
            dinv_c[:BW[b], b] = dv[b * 128:b * 128 + BW[b]]
        bl = batch[c * SH:(c + 1) * SH]
        g0 = int(bl[0])
        pooled_base[c] = g0
        brel = np.full((128, NBLK), -1.0, np.float32)
        for b in range(NBLK):
            rel = (bl[b * 128:b * 128 + BW[b]] - g0 - ws_blk[b]).astype(np.int64)
            assert rel.min() >= 0 and rel.max() < 128, (c, b, rel.min(), rel.max())
            brel[:BW[b], b] = rel.astype(np.float32)
        iota = np.broadcast_to(np.arange(128, dtype=np.float32), (128, 128)).copy()
        iota3 = np.tile(np.arange(128, dtype=np.float32), (128, WCH)).copy()
        Wk = np.ascontiguousarray(Ws.transpose(1, 0, 2).reshape(128, L * 128))
        bias_b = np.ascontiguousarray(
            np.broadcast_to(bs[:, None, :], (L, 128, 128)).transpose(1, 0, 2).reshape(128, L * 128))
        ins_per_core.append({
            "xT": xT, "Wk": Wk, "biasb": bias_b, "dinvc": dinv_c, "brel": brel,
            "iota": iota, "hw": head_w.reshape(128, 1).astype(np.float32),
            "idx": idx_sb.astype(np.int16),
            "dl": dl_sb.astype(np.float32),
            "iota3": iota3,
        })
    struct = {
        "NW": NW, "NWmax": NWmax, "CTOT": CTOT, "idx_cols": idx_cols,
        "qwin_base": qwin_base, "blk_chunks": blk_chunks, "blk_ready": blk_ready,
        "ws_blk": ws_blk, "pooled_base": pooled_base,
        "head_b": float(np.asarray(head_b).reshape(-1)[0]),
    }
    return ins_per_core, struct


def _build(struct):
    import concourse.bass as bass
    import concourse.bacc as bacc
    import concourse.mybir as mybir
    import concourse.tile as tile
    from concourse.masks import make_identity

    NW = struct["NW"]
    qwin_base = struct["qwin_base"]
    blk_chunks = struct["blk_chunks"]
    blk_ready = struct["blk_ready"]
    ws_blk = struct["ws_blk"]
    idx_cols = struct["idx_cols"]
    CTOT = struct["CTOT"]
    f32 = mybir.dt.float32
    bf16 = mybir.dt.bfloat16

    nc = bacc.Bacc("TRN2", target_bir_lowering=False, debug=False,
                   num_devices=NC, num_swdge_queues=4)
    xT_d = nc.dram_tensor("xT", [128, NBLK * 128], f32, kind="ExternalInput")
    Wk_d = nc.dram_tensor("Wk", [128, L * 128], f32, kind="ExternalInput")
    bias_d = nc.dram_tensor("biasb", [128, L * 128], f32, kind="ExternalInput")
    dinv_d = nc.dram_tensor("dinvc", [128, NBLK], f32, kind="ExternalInput")
    brel_d = nc.dram_tensor("brel", [128, NBLK], f32, kind="ExternalInput")
    iota_d = nc.dram_tensor("iota", [128, 128], f32, kind="ExternalInput")
    iota3_d = nc.dram_tensor("iota3", [128, WCH * 128], f32, kind="ExternalInput")
    hw_d = nc.dram_tensor("hw", [128, 1], f32, kind="ExternalInput")
    idx_d = nc.dram_tensor("idx", [128, idx_cols], mybir.dt.int16, kind="ExternalInput")
    dl_d = nc.dram_tensor("dl", [128, CTOT], f32, kind="ExternalInput")
    out_d = nc.dram_tensor("out", [1, POOLW], f32, kind="ExternalOutput")

    with tile.TileContext(nc) as tc:
        with (
            tc.tile_pool(name="const", bufs=1) as cp,
            tc.tile_pool(name="hT", bufs=2) as htp,
            tc.tile_pool(name="m0", bufs=MSG_BUFS) as mp0,
            tc.tile_pool(name="m1", bufs=MSG_BUFS) as mp1,
            tc.tile_pool(name="m2", bufs=MSG_BUFS) as mp2,
            tc.tile_pool(name="m3", bufs=MSG_BUFS) as mp3,
            tc.tile_pool(name="s0", bufs=S_BUFS) as sp0,
            tc.tile_pool(name="s1", bufs=S_BUFS) as sp1,
            tc.tile_pool(name="s2", bufs=S_BUFS) as sp2,
            tc.tile_pool(name="s3", bufs=S_BUFS) as sp3,
            tc.tile_pool(name="ev", bufs=3) as evp,
            tc.tile_pool(name="tv", bufs=3) as tvp,
            tc.tile_pool(name="psX", bufs=3, space="PSUM") as psX,
            tc.tile_pool(name="psB", bufs=4, space="PSUM") as psB,
            tc.tile_pool(name="psH", bufs=1, space="PSUM") as psH,
            tc.tile_pool(name="dram", bufs=1, space="DRAM") as dp,
        ):
            mpools = [mp0, mp1, mp2, mp3]
            spools = [sp0, sp1, sp2, sp3]
            # constants
            Wk = cp.tile([128, L * 128], f32)
            nc.sync.dma_start(Wk[:], Wk_d[:])
            biasb = cp.tile([128, L * 128], f32)
            nc.sync.dma_start(biasb[:], bias_d[:])
            dinvc = cp.tile([128, NBLK], f32)
            nc.sync.dma_start(dinvc[:], dinv_d[:])
            brel = cp.tile([128, NBLK], f32)
            nc.sync.dma_start(brel[:], brel_d[:])
            iota = cp.tile([128, 128], f32)
            nc.sync.dma_start(iota[:], iota_d[:])
            iota3 = cp.tile([128, WCH * 128], f32)
            nc.sync.dma_start(iota3[:], iota3_d[:])
            hw = cp.tile([128, 1], f32)
            nc.sync.dma_start(hw[:], hw_d[:])
            idxt = cp.tile([128, idx_cols], mybir.dt.int16)
            nc.sync.dma_start(idxt[:], idx_d[:])
            dlt = cp.tile([128, CTOT], f32)
            nc.sync.dma_start(dlt[:], dl_d[:])
            ident = cp.tile([128, 128], f32)
            make_identity(nc, ident[:])
            pooledT = cp.tile([128, POOLW], f32)
            nc.vector.memset(pooledT[:], 0.0)

            # agin (broadcast input) slices: A for b_0/b_2, B for b_1
            aginA = [dp.tile([QROWS[q], 128], bf16, name=f"aginA{q}") for q in range(NQ)]
            aginB = [dp.tile([QROWS[q], 128], bf16, name=f"aginB{q}") for q in range(NQ)]
            agin_for_layer = [aginA, aginB, aginA]
            agouts = [[dp.tile([NC * QROWS[q], 128], bf16, name=f"agout{l}_{q}",
                               addr_space="Shared") for q in range(NQ)] for l in range(L)]

            def fire_ag(l, q):
                agin = agin_for_layer[l][q]
                nc.gpsimd.collective_compute(
                    "AllGather", mybir.AluOpType.bypass,
                    ins=[agin.opt()], outs=[agouts[l][q].opt()],
                    replica_groups=[list(range(NC))],
                )

            def write_agin(l, b, tev, w):
                # tev [w,128] bf16 = (h @ W_l)*dinv rows of block b -> agin slice
                q = BLK_Q[b]
                r0 = b * 128 - QROWBASE[q]
                nc.sync.dma_start(agin_for_layer[l][q][r0:r0 + w, :], tev[0:w, :])

            # ---------- startup: b_0 = (x @ W_0) * dinv ----------
            for hc in range(7):
                cols = slice(hc * 1792, (hc + 1) * 1792)
                hTt = htp.tile([128, 1792], f32, tag="hT")
                nc.sync.dma_start(hTt[:], xT_d[:, cols])
                for bi in range(14):
                    b = hc * 14 + bi
                    w = BW[b]
                    pt = psX.tile([128, 128], f32, tag="psX")
                    nc.tensor.matmul(pt[0:w, :], lhsT=hTt[:, bi * 128:bi * 128 + w],
                                     rhs=Wk[:, 0:128], start=True, stop=True)
                    tev = tvp.tile([128, 128], bf16, tag="tev")
                    nc.vector.tensor_scalar_mul(tev[0:w, :], pt[0:w, :],
                                                dinvc[0:w, b:b + 1])
                    write_agin(0, b, tev, w)
                    if b in AG_FIRE_BLOCKS:
                        fire_ag(0, AG_FIRE_BLOCKS[b])

            for l in range(L):
                # ---------- phase B: gather + segment-sum matmuls ----------
                mtiles = {}
                stiles = {}
                emitted = 0

                def emit_block(b, l=l):
                    w = BW[b]
                    refs = blk_chunks[b]
                    pa = psB.tile([128, 128], f32, tag="agg")
                    for i, (qq, ww, ss) in enumerate(refs):
                        nc.tensor.matmul(
                            pa[:], lhsT=stiles[(qq, ww)][:, ss, :],
                            rhs=mtiles[(qq, ww)][:, ss, :],
                            start=(i == 0), stop=(i == len(refs) - 1))
                    hs = evp.tile([128, 128], f32, tag="hs")
                    nc.vector.tensor_scalar_mul(hs[0:w, :], pa[0:w, :], dinvc[0:w, b:b + 1])
                    hs2 = evp.tile([128, 128], f32, tag="hs2")
                    nc.vector.tensor_tensor(out=hs2[0:w, :], in0=hs[0:w, :],
                                            in1=biasb[0:w, l * 128:(l + 1) * 128],
                                            op=mybir.AluOpType.add)
                    hs3 = evp.tile([128, 128], f32, tag="hs3")
                    nc.scalar.activation(hs3[0:w, :], hs2[0:w, :],
                                         mybir.ActivationFunctionType.Relu)
                    if l < 2:
                        # fused next-layer transform: b_{l+1} rows for block b
                        ptr = psX.tile([128, 128], f32, tag="psX")
                        nc.tensor.transpose(ptr[:], hs3[:], ident[:])
                        hTs = evp.tile([128, 128], f32, tag="hTs")
                        nc.vector.tensor_copy(hTs[:], ptr[:])
                        pt2 = psX.tile([128, 128], f32, tag="psX")
                        nc.tensor.matmul(pt2[0:w, :], lhsT=hTs[:, 0:w],
                                         rhs=Wk[:, (l + 1) * 128:(l + 2) * 128],
                                         start=True, stop=True)
                        tev = tvp.tile([128, 128], bf16, tag="tev")
                        nc.vector.tensor_scalar_mul(tev[0:w, :], pt2[0:w, :],
                                                    dinvc[0:w, b:b + 1])
                        write_agin(l + 1, b, tev, w)
                        if b in AG_FIRE_BLOCKS:
                            fire_ag(l + 1, AG_FIRE_BLOCKS[b])
                    else:
                        spool_t = evp.tile([128, 128], f32, tag="spool")
                        nc.vector.tensor_tensor(
                            out=spool_t[:], in0=brel[:, b:b + 1].to_broadcast([128, 128]),
                            in1=iota[:], op=mybir.AluOpType.is_equal)
                        pp = psX.tile([128, 128], f32, tag="psX")
                        nc.tensor.matmul(pp[:], lhsT=hs3[:], rhs=spool_t[:],
                                         start=True, stop=True)
                        wsb = ws_blk[b]
                        nc.vector.tensor_tensor(
                            out=pooledT[:, wsb:wsb + 128], in0=pooledT[:, wsb:wsb + 128],
                            in1=pp[:], op=mybir.AluOpType.add)

                for ww in range(struct["NWmax"]):
                    for qq in range(NQ):
                        if ww >= NW[qq]:
                            continue
                        g = mpools[qq].tile([128, WCH, 128], bf16, tag=f"msg{qq}")
                        icol = (int(qwin_base[qq]) + ww) * (NIDX // 16)
                        nc.gpsimd.dma_gather(
                            out_ap=g[:],
                            in_ap=agouts[l][qq][:],
                            idxs_ap=idxt[:, icol:icol + NIDX // 16],
                            num_idxs=NIDX, num_idxs_reg=NIDX, elem_size=128,
                            single_packet=False, queue_num=qq)
                        mtiles[(qq, ww)] = g
                        st = spools[qq].tile([128, WCH, 128], bf16, tag=f"S{qq}")
                        dcol = (int(qwin_base[qq]) + ww) * WCH
                        nc.vector.tensor_tensor(
                            out=st[:],
                            in0=dlt[:, dcol:dcol + WCH].to_broadcast([128, WCH, 128]),
                            in1=iota3[:].rearrange("p (w d) -> p w d", w=WCH),
                            op=mybir.AluOpType.is_equal)
                        stiles[(qq, ww)] = st
                    while emitted < NBLK and blk_ready[emitted] <= ww:
                        emit_block(emitted)
                        emitted += 1
                while emitted < NBLK:
                    emit_block(emitted)
                    emitted += 1

            # ---------- head: partial logits ----------
            ph = psH.tile([128, POOLW], f32)
            nc.tensor.matmul(ph[0:1, :], lhsT=hw[:, 0:1], rhs=pooledT[:],
                             start=True, stop=True)
            outsb = cp.tile([1, POOLW], f32)
            nc.vector.tensor_copy(outsb[:], ph[0:1, :])
            nc.sync.dma_start(out_d[:], outsb[:])
    nc.compile()
    return nc


# ---------------------------------------------------------------------------
# PJRT compile-once runner (inlined; mirrors concourse.bass2jax.run_bass_via_pjrt)
# ---------------------------------------------------------------------------
class _Runner:
    def __init__(self, nc, n_cores):
        import jax
        import numpy as np
        from jax.sharding import Mesh, PartitionSpec
        from jax.experimental.shard_map import shard_map
        import concourse.mybir as mybir
        from concourse import bass2jax
        from concourse.bass2jax import _bass_exec_p, partition_id_tensor

        bass2jax.install_neuronx_cc_hook()
        self.jax = jax
        self.nc = nc
        self.n_cores = n_cores
        partition_name = nc.partition_id_tensor.name if nc.partition_id_tensor else None
        in_names, out_names, out_avals, zero_outs = [], [], [], []
        for alloc in nc.m.functions[0].allocations:
            if not isinstance(alloc, mybir.MemoryLocationSet):
                continue
            name = alloc.memorylocations[0].name
            if alloc.kind == "ExternalInput":
                if name != partition_name:
                    in_names.append(name)
            elif alloc.kind == "ExternalOutput":
                out_names.append(name)
                out_avals.append(jax.core.ShapedArray(tuple(alloc.tensor_shape),
                                                      mybir.dt.np(alloc.dtype)))
                zero_outs.append(np.zeros(tuple(alloc.tensor_shape),
                                          mybir.dt.np(alloc.dtype)))
        self.in_names, self.out_names = in_names, out_names
        self.out_avals, self.zero_outs = out_avals, zero_outs
        n_params, n_outs = len(in_names), len(out_avals)
        all_in = list(in_names) + list(out_names)
        if partition_name is not None:
            all_in.append(partition_name)

        def _body(*args):
            operands = list(args)
            if partition_name is not None:
                operands.append(partition_id_tensor())
            return tuple(_bass_exec_p.bind(
                *operands, out_avals=tuple(out_avals), in_names=tuple(all_in),
                out_names=tuple(out_names), lowering_input_output_aliases=(),
                sim_require_finite=False, sim_require_nnan=False, nc=nc))

        devices = jax.devices()[:n_cores]
        self.mesh = Mesh(np.asarray(devices), ("core",))
        in_specs = (PartitionSpec("core"),) * (n_params + n_outs)
        out_specs = (PartitionSpec("core"),) * n_outs
        self.sharded = jax.jit(
            shard_map(_body, mesh=self.mesh, in_specs=in_specs,
                      out_specs=out_specs, check_rep=False),
            donate_argnums=tuple(range(n_params, n_params + n_outs)),
            keep_unused=True)

    def run(self, in_maps):
        import numpy as np
        from jax.sharding import NamedSharding, PartitionSpec
        sharding = NamedSharding(self.mesh, PartitionSpec("core"))
        concat = [self.jax.device_put(
            np.concatenate([np.asarray(in_maps[c][n]) for c in range(self.n_cores)], axis=0),
            sharding) for n in self.in_names]
        zeros = [self.jax.device_put(
            np.zeros((self.n_cores * z.shape[0], *z.shape[1:]), z.dtype), sharding)
            for z in self.zero_outs]
        outs = self.sharded(*concat, *zeros)
        self.jax.block_until_ready(outs)
        return [
            {n: np.asarray(outs[i]).reshape(self.n_cores, *self.out_avals[i].shape)[c]
             for i, n in enumerate(self.out_names)}
            for c in range(self.n_cores)
        ]


_CACHE = {}


def kernel(x, edge_index, batch, Ws, bs, head_w, head_b):
    import hashlib
    ins_per_core, struct = _prep(x, edge_index, batch, Ws, bs, head_w, head_b)
    h = hashlib.sha1()
    h.update(np.ascontiguousarray(edge_index).tobytes())
    h.update(np.ascontiguousarray(batch).tobytes())
    key = h.hexdigest()
    if key not in _CACHE:
        nc = _build(struct)
        _CACHE[key] = _Runner(nc, NC)
        _CACHE["gcn"] = _CACHE[key]
    runner = _CACHE[key]
    results = runner.run(ins_per_core)
    out = np.zeros(G, np.float64)
    for c in range(NC):
        part = results[c]["out"].reshape(-1)
        g0 = int(struct["pooled_base"][c])
        w = min(POOLW, G - g0)
        out[g0:g0 + w] += part[:w]
    out += struct["head_b"]
    return out.astype(np.float32)


# revision 16
# speedup vs baseline: 13.0933x; 13.0933x over previous
"""SimpleGCN (3-layer GCNConv + global_add_pool + linear head) on 8 Trainium2 cores.

v3 strategy (shapes hardcoded for nn_SimpleGCN):
 - Nodes sharded contiguously across 8 cores by dst (12500 each).
 - Broadcast value per layer l: b_l = (h_{l-1} @ W_l) * dinv (bf16, 128 cols).
   b_0 from a startup pass over x; b_{l+1} fused into layer l's per-block
   epilogue (transpose -> W matmul -> dinv scale -> DMA into agin slice).
 - agin split into 4 block-aligned slices (27/27/27/17 blocks); each slice
   AllGathers as soon as its last block is written, overlapping phase B.
 - Self-loop terms are NOT gathered: emit injects them with an identity
   matmul against the block's own b_l rows (loaded from agin_l).
 - Non-loop edges bucketed per (dst-block, src-slice-quarter), packed TIGHTLY
   (no per-block chunk roundup) into per-(group of 7 blocks, quarter) gather
   windows sized to the max core's count; per-core trailing slots get idx=-1,
   which the SWDGE ucode trims before generating descriptors (free padding).
 - Chunks straddling two blocks are consumed by both blocks' matmul chains;
   per-pair dst-local columns (dl) select only the owning block's rows via
   is_equal one-hots built per block in a single DVE op (bf16).
 - PE accumulates each block's segment sum in PSUM; epilogue relu etc as v2.
"""
import math
import numpy as np

N_NODES = 100000
N_EDGES = 1600000
D = 128
L = 3
G = 512
NC = 8
SH = N_NODES // NC            # 12500 nodes per core
NBLK = math.ceil(SH / 128)    # 98 blocks (97 full + one of 84)
BW = [128] * (NBLK - 1) + [SH - 128 * (NBLK - 1)]
NQ = 4
QBLK = [27, 27, 27, 17]                      # blocks per slice-quarter
QSTART = [0, 27, 54, 81]
QROWS = [3456, 3456, 3456, SH - 81 * 128]    # rows per quarter (last: 2132)
QROWBASE = [0, 3456, 6912, 10368]
K = 7                                        # blocks per gather window group
NGRP = -(-NBLK // K)                         # 14 groups
MSG_BUFS = 2
S_BUFS = 2
POOLW = 256                   # per-core local pooled window

BLK_Q = [min(b // 27, 3) for b in range(NBLK)]
AG_FIRE_BLOCKS = {QSTART[q] + QBLK[q] - 1: q for q in range(NQ)}  # {26:0,53:1,80:2,97:3}


def _prep(x, edge_index, batch, Ws, bs, head_w, head_b):
    x = np.asarray(x, np.float32)
    ei = np.asarray(edge_index, np.int64)
    batch = np.asarray(batch, np.int64)
    Ws = np.asarray(Ws, np.float32)
    bs = np.asarray(bs, np.float32)
    head_w = np.asarray(head_w, np.float32)

    src, dst = ei[0], ei[1]
    # degree includes the self-loops the reference adds
    deg = (np.bincount(np.concatenate([dst, np.arange(N_NODES, dtype=np.int64)]),
                       minlength=N_NODES)).astype(np.float32)
    dinv = np.where(deg > 0, 1.0 / np.sqrt(deg), 0.0).astype(np.float32)

    qrows = np.asarray(QROWS, np.int64)
    qbase = np.asarray(QROWBASE, np.int64)
    s_off = src % SH
    s_q = np.minimum(s_off // 3456, 3)
    s_idx = (src // SH) * qrows[s_q] + (s_off - qbase[s_q])

    core = dst // SH
    per_core = []
    counts = np.zeros((NC, NQ, NBLK), np.int64)
    for c in range(NC):
        m = core == c
        si_c = s_idx[m]
        dloc = dst[m] - c * SH
        b = dloc >> 7
        key = s_q[m] * NBLK + b
        order = np.argsort(key, kind="stable")
        counts[c] = np.bincount(key, minlength=NQ * NBLK).reshape(NQ, NBLK)
        per_core.append((si_c[order], dloc[order],
                         np.concatenate([[0], np.cumsum(counts[c].reshape(-1))])))

    # ---- window (q,g) budgets: tight pack, max over cores ----
    Cgq = np.zeros((NQ, NGRP), np.int64)
    for q in range(NQ):
        for g in range(NGRP):
            lo, hi = g * K, min((g + 1) * K, NBLK)
            Cgq[q, g] = max(1, int((-(-counts[:, q, lo:hi].sum(axis=1).max() // 128))))
    MAXG = int(Cgq.max())

    # window order (g-major), idx column base, slot base
    win_order = [(g, q) for g in range(NGRP) for q in range(NQ)]
    wbase = {}
    sbase = {}
    run_i = 0
    run_s = 0
    for (g, q) in win_order:
        wbase[(q, g)] = run_i
        sbase[(q, g)] = run_s
        run_i += int(Cgq[q, g]) * 8
        run_s += int(Cgq[q, g]) * 128
    idx_cols = run_i
    TOTSLOT = run_s

    # per-core tight layout offsets + program-level pair spans
    # cum_off[c, q, b] = slot offset of block b's run inside window (q, b//K)
    cum_off = np.zeros((NC, NQ, NBLK), np.int64)
    for q in range(NQ):
        for g in range(NGRP):
            lo, hi = g * K, min((g + 1) * K, NBLK)
            for c in range(NC):
                run = 0
                for b in range(lo, hi):
                    cum_off[c, q, b] = run
                    run += counts[c, q, b]
    # pairs grouped by block: (q, g, ch) spans = union over cores
    pair_list = [[] for _ in range(NBLK)]
    for b in range(NBLK):
        g = b // K
        for q in range(NQ):
            c0 = min(int(cum_off[c, q, b]) // 128 for c in range(NC))
            c1 = max(int(cum_off[c, q, b] + counts[c, q, b] - 1) // 128
                     if counts[c, q, b] > 0 else int(cum_off[c, q, b]) // 128
                     for c in range(NC))
            for ch in range(c0, c1 + 1):
                pair_list[b].append((q, g, ch))
    np_b = [len(pl) for pl in pair_list]
    MAXNP = max(np_b)
    pair_base = np.concatenate([[0], np.cumsum(np_b)]).astype(np.int64)
    TOTPAIR = int(pair_base[-1])

    ins_per_core = []
    pooled_base = np.zeros(NC, np.int64)
    ws_blk = [max(0, int(b * 128 * G / N_NODES) - 32) for b in range(NBLK)]
    # NOTE: negative (trimmable) pad indices hang the device — the decode-side
    # ring reservation desyncs from the ucode's trimmed push count. Use idx=0
    # pads; their rows are masked out by dl=-1 in the S one-hots.
    for c in range(NC):
        si_c, dloc, cstarts = per_core[c]
        ixf = np.full(TOTSLOT, 0, np.int64)
        dlp = np.full((TOTPAIR, 128), -1.0, np.float32)    # per-pair dst-local cols
        for q in range(NQ):
            for b in range(NBLK):
                n = int(counts[c, q, b])
                if n == 0:
                    continue
                g = b // K
                st = int(cstarts[q * NBLK + b])
                s0 = sbase[(q, g)] + int(cum_off[c, q, b])
                ixf[s0:s0 + n] = si_c[st:st + n]
        for b in range(NBLK):
            for j, (q, g, ch) in enumerate(pair_list[b]):
                pcol = int(pair_base[b]) + j
                s0 = sbase[(q, g)] + ch * 128
                # rows of this chunk (on this core) belonging to block b
                st = int(cstarts[q * NBLK + b])
                n = int(counts[c, q, b])
                r0 = sbase[(q, g)] + int(cum_off[c, q, b])
                lo = max(s0, r0)
                hi = min(s0 + 128, r0 + n)
                if hi > lo:
                    dlp[pcol, lo - s0:hi - s0] = (dloc[st + (lo - r0):st + (hi - r0)] % 128
                                                  ).astype(np.float32)
        # wrap indices per window: slot j -> idx[p=j%16, wbase + j//16], replicated x8
        idx_sb = np.zeros((128, idx_cols), np.int16)
        for (g, q) in win_order:
            Cw = int(Cgq[q, g])
            w0, s0 = wbase[(q, g)], sbase[(q, g)]
            blkix = ixf[s0:s0 + Cw * 128].reshape(Cw * 8, 16).T.astype(np.int16)  # [16, Cw*8]
            idx_sb[:, w0:w0 + Cw * 8] = np.tile(blkix, (8, 1))
        dl_sb = dlp.T.copy()                               # [128, TOTPAIR]

        xT = np.zeros((128, NBLK * 128), np.float32)
        xT[:, :SH] = x[c * SH:(c + 1) * SH].T
        dinv_c = np.ones((128, NBLK), np.float32)
        dv = dinv[c * SH:(c + 1) * SH]
        for b in range(NBLK):
            dinv_c[:BW[b], b] = dv[b * 128:b * 128 + BW[b]]
        bl = batch[c * SH:(c + 1) * SH]
        g0 = int(bl[0])
        pooled_base[c] = g0
        brel = np.full((128, NBLK), -1.0, np.float32)
        for b in range(NBLK):
            rel = (bl[b * 128:b * 128 + BW[b]] - g0 - ws_blk[b]).astype(np.int64)
            assert rel.min() >= 0 and rel.max() < 128, (c, b, rel.min(), rel.max())
            brel[:BW[b], b] = rel.astype(np.float32)
        iota = np.broadcast_to(np.arange(128, dtype=np.float32), (128, 128)).copy()
        iota3 = np.tile(np.arange(128, dtype=np.float32), (128, MAXNP)).copy()
        Wk = np.ascontiguousarray(Ws.transpose(1, 0, 2).reshape(128, L * 128))
        bias_b = np.ascontiguousarray(
            np.broadcast_to(bs[:, None, :], (L, 128, 128)).transpose(1, 0, 2).reshape(128, L * 128))
        import ml_dtypes
        ins_per_core.append({
            "xT": xT, "Wk": Wk, "biasb": bias_b, "dinvc": dinv_c, "brel": brel,
            "iota": iota, "hw": head_w.reshape(128, 1).astype(np.float32),
            "idx": idx_sb,
            "dl": dl_sb.astype(ml_dtypes.bfloat16),
            "iota3": iota3.astype(ml_dtypes.bfloat16),
        })
    struct = {
        "Cgq": Cgq, "MAXG": MAXG, "MAXNP": MAXNP, "idx_cols": idx_cols,
        "wbase": wbase, "pair_list": pair_list, "pair_base": pair_base,
        "TOTPAIR": TOTPAIR, "ws_blk": ws_blk, "pooled_base": pooled_base,
        "bs_nonzero": bool(np.any(bs != 0)),
        "head_b": float(np.asarray(head_b).reshape(-1)[0]),
    }
    return ins_per_core, struct


def _build(struct):
    import concourse.bass as bass
    import concourse.bacc as bacc
    import concourse.mybir as mybir
    import concourse.tile as tile
    from concourse.masks import make_identity

    Cgq = struct["Cgq"]
    wbase = struct["wbase"]
    pair_list = struct["pair_list"]
    pair_base = struct["pair_base"]
    TOTPAIR = struct["TOTPAIR"]
    MAXG = struct["MAXG"]
    MAXNP = struct["MAXNP"]
    idx_cols = struct["idx_cols"]
    ws_blk = struct["ws_blk"]
    f32 = mybir.dt.float32
    bf16 = mybir.dt.bfloat16

    nc = bacc.Bacc("TRN2", target_bir_lowering=False, debug=False,
                   num_devices=NC, num_swdge_queues=4,
                   dynamic_dma_scratch_size=36864)
    xT_d = nc.dram_tensor("xT", [128, NBLK * 128], f32, kind="ExternalInput")
    Wk_d = nc.dram_tensor("Wk", [128, L * 128], f32, kind="ExternalInput")
    bias_d = nc.dram_tensor("biasb", [128, L * 128], f32, kind="ExternalInput")
    dinv_d = nc.dram_tensor("dinvc", [128, NBLK], f32, kind="ExternalInput")
    brel_d = nc.dram_tensor("brel", [128, NBLK], f32, kind="ExternalInput")
    iota_d = nc.dram_tensor("iota", [128, 128], f32, kind="ExternalInput")
    iota3_d = nc.dram_tensor("iota3", [128, MAXNP * 128], bf16, kind="ExternalInput")
    hw_d = nc.dram_tensor("hw", [128, 1], f32, kind="ExternalInput")
    idx_d = nc.dram_tensor("idx", [128, idx_cols], mybir.dt.int16, kind="ExternalInput")
    dl_d = nc.dram_tensor("dl", [128, TOTPAIR], bf16, kind="ExternalInput")
    out_d = nc.dram_tensor("out", [1, POOLW], f32, kind="ExternalOutput")

    with tile.TileContext(nc) as tc:
        with (
            tc.tile_pool(name="const", bufs=1) as cp,
            tc.tile_pool(name="hT", bufs=2) as htp,
            tc.tile_pool(name="m0", bufs=4) as mp0,
            tc.tile_pool(name="m1", bufs=3) as mp1,
            tc.tile_pool(name="m2", bufs=3) as mp2,
            tc.tile_pool(name="m3", bufs=2) as mp3,
            tc.tile_pool(name="sb", bufs=S_BUFS) as sbp,
            tc.tile_pool(name="sl", bufs=3) as slp,
            tc.tile_pool(name="ev", bufs=3) as evp,
            tc.tile_pool(name="tv", bufs=3) as tvp,
            tc.tile_pool(name="psX", bufs=3, space="PSUM") as psX,
            tc.tile_pool(name="psB", bufs=4, space="PSUM") as psB,
            tc.tile_pool(name="psH", bufs=1, space="PSUM") as psH,
            tc.tile_pool(name="dram", bufs=1, space="DRAM") as dp,
        ):
            mpools = [mp0, mp1, mp2, mp3]
            # constants
            Wk = cp.tile([128, L * 128], f32)
            nc.sync.dma_start(Wk[:], Wk_d[:])
            biasb = cp.tile([128, L * 128], f32)
            nc.sync.dma_start(biasb[:], bias_d[:])
            dinvc = cp.tile([128, NBLK], f32)
            nc.sync.dma_start(dinvc[:], dinv_d[:])
            brel = cp.tile([128, NBLK], f32)
            nc.sync.dma_start(brel[:], brel_d[:])
            iota = cp.tile([128, 128], f32)
            nc.sync.dma_start(iota[:], iota_d[:])
            iota3 = cp.tile([128, MAXNP * 128], bf16)
            nc.sync.dma_start(iota3[:], iota3_d[:])
            hw = cp.tile([128, 1], f32)
            nc.sync.dma_start(hw[:], hw_d[:])
            idxt = cp.tile([128, idx_cols], mybir.dt.int16)
            nc.sync.dma_start(idxt[:], idx_d[:])
            dlt = cp.tile([128, TOTPAIR], bf16)
            nc.sync.dma_start(dlt[:], dl_d[:])
            ident = cp.tile([128, 128], f32)
            make_identity(nc, ident[:])
            identb = cp.tile([128, 128], bf16)
            nc.vector.tensor_copy(identb[:], ident[:])
            pooledT = cp.tile([128, POOLW], f32)
            nc.vector.memset(pooledT[:], 0.0)

            # NaN guard: trailing window slots are skipped by the gather ucode,
            # so pre-zero every msg/self pool buffer once (physical bufs cycle).
            for qq, pool in enumerate((mp0, mp1, mp2, mp3)):
                for _ in range((4, 3, 3, 2)[qq]):
                    z = pool.tile([128, MAXG, 128], bf16, tag=f"msg{qq}")
                    nc.vector.memset(z[:], 0.0)
            for _ in range(3):
                z = slp.tile([128, 128], bf16, tag="sl")
                nc.vector.memset(z[:], 0.0)

            agins = [[dp.tile([QROWS[q], 128], bf16, name=f"agin{l}_{q}")
                      for q in range(NQ)] for l in range(L)]
            agouts = [[dp.tile([NC * QROWS[q], 128], bf16, name=f"agout{l}_{q}",
                               addr_space="Shared") for q in range(NQ)] for l in range(L)]

            def fire_ag(l, q):
                nc.gpsimd.collective_compute(
                    "AllGather", mybir.AluOpType.bypass,
                    ins=[agins[l][q].opt()], outs=[agouts[l][q].opt()],
                    replica_groups=[list(range(NC))],
                )

            def write_agin(l, b, tev, w):
                q = BLK_Q[b]
                r0 = b * 128 - QROWBASE[q]
                nc.sync.dma_start(agins[l][q][r0:r0 + w, :], tev[0:w, :])

            # ---------- startup: b_0 = (x @ W_0) * dinv ----------
            for hc in range(7):
                cols = slice(hc * 1792, (hc + 1) * 1792)
                hTt = htp.tile([128, 1792], f32, tag="hT")
                nc.sync.dma_start(hTt[:], xT_d[:, cols])
                for bi in range(14):
                    b = hc * 14 + bi
                    w = BW[b]
                    pt = psX.tile([128, 128], f32, tag="psX")
                    nc.tensor.matmul(pt[0:w, :], lhsT=hTt[:, bi * 128:bi * 128 + w],
                                     rhs=Wk[:, 0:128], start=True, stop=True)
                    tev = tvp.tile([128, 128], bf16, tag="tev")
                    nc.scalar.activation(tev[0:w, :], pt[0:w, :],
                                         mybir.ActivationFunctionType.Copy,
                                         scale=dinvc[0:w, b:b + 1])
                    write_agin(0, b, tev, w)
                    if b in AG_FIRE_BLOCKS:
                        fire_ag(0, AG_FIRE_BLOCKS[b])

            _qrr = [0]
            for l in range(L):
                mtiles = {}

                def emit_block(b, l=l):
                    w = BW[b]
                    npb = int(pair_base[b + 1] - pair_base[b])
                    pb = int(pair_base[b])
                    # self-loop rows: own block's b_l rows from agin
                    q0 = BLK_Q[b]
                    r0 = b * 128 - QROWBASE[q0]
                    sl = slp.tile([128, 128], bf16, tag="sl")
                    nc.sync.dma_start(sl[0:w, :], agins[l][q0][r0:r0 + w, :])
                    # one-hot S for all of this block's chunk pairs
                    S_b = sbp.tile([128, MAXNP, 128], bf16, tag="S")
                    nc.vector.tensor_tensor(
                        out=S_b[:, 0:npb, :],
                        in0=dlt[:, pb:pb + npb].to_broadcast([128, npb, 128]),
                        in1=iota3[:, 0:npb * 128].rearrange("p (n d) -> p n d", n=npb),
                        op=mybir.AluOpType.is_equal)
                    pa = psB.tile([128, 128], f32, tag="agg")
                    nc.tensor.matmul(pa[:], lhsT=identb[:], rhs=sl[:],
                                     start=True, stop=False)
                    for j, (qq, gg, ch) in enumerate(pair_list[b]):
                        nc.tensor.matmul(
                            pa[:], lhsT=S_b[:, j, :], rhs=mtiles[(qq, gg)][:, ch, :],
                            start=False, stop=(j == npb - 1))
                    hs3 = evp.tile([128, 128], f32, tag="hs3")
                    if struct["bs_nonzero"]:
                        hs = evp.tile([128, 128], f32, tag="hs")
                        nc.vector.tensor_scalar_mul(hs[0:w, :], pa[0:w, :], dinvc[0:w, b:b + 1])
                        hs2 = evp.tile([128, 128], f32, tag="hs2")
                        nc.vector.tensor_tensor(out=hs2[0:w, :], in0=hs[0:w, :],
                                                in1=biasb[0:w, l * 128:(l + 1) * 128],
                                                op=mybir.AluOpType.add)
                        nc.scalar.activation(hs3[0:w, :], hs2[0:w, :],
                                             mybir.ActivationFunctionType.Relu)
                    else:
                        nc.scalar.activation(hs3[0:w, :], pa[0:w, :],
                                             mybir.ActivationFunctionType.Relu,
                                             scale=dinvc[0:w, b:b + 1])
                    if l < 2:
                        ptr = psX.tile([128, 128], f32, tag="psX")
                        nc.tensor.transpose(ptr[:], hs3[:], ident[:])
                        hTs = evp.tile([128, 128], f32, tag="hTs")
                        nc.scalar.activation(hTs[:], ptr[:],
                                             mybir.ActivationFunctionType.Copy)
                        pt2 = psX.tile([128, 128], f32, tag="psX")
                        nc.tensor.matmul(pt2[0:w, :], lhsT=hTs[:, 0:w],
                                         rhs=Wk[:, (l + 1) * 128:(l + 2) * 128],
                                         start=True, stop=True)
                        tev = tvp.tile([128, 128], bf16, tag="tev")
                        nc.scalar.activation(tev[0:w, :], pt2[0:w, :],
                                             mybir.ActivationFunctionType.Copy,
                                             scale=dinvc[0:w, b:b + 1])
                        write_agin(l + 1, b, tev, w)
                        if b in AG_FIRE_BLOCKS:
                            fire_ag(l + 1, AG_FIRE_BLOCKS[b])
                    else:
                        spool_t = evp.tile([128, 128], f32, tag="spool")
                        nc.vector.tensor_tensor(
                            out=spool_t[:], in0=brel[:, b:b + 1].to_broadcast([128, 128]),
                            in1=iota[:], op=mybir.AluOpType.is_equal)
                        pp = psX.tile([128, 128], f32, tag="psX")
                        nc.tensor.matmul(pp[:], lhsT=hs3[:], rhs=spool_t[:],
                                         start=True, stop=True)
                        wsb = ws_blk[b]
                        nc.vector.tensor_tensor(
                            out=pooledT[:, wsb:wsb + 128], in0=pooledT[:, wsb:wsb + 128],
                            in1=pp[:], op=mybir.AluOpType.add)

                for s_ in range(NGRP + NQ - 1):
                    for qq in range(NQ):
                        g = s_ - qq
                        if not (0 <= g < NGRP):
                            continue
                        Cw = int(Cgq[qq, g])
                        gt = mpools[qq].tile([128, MAXG, 128], bf16, tag=f"msg{qq}")
                        w0 = wbase[(qq, g)]
                        nc.gpsimd.dma_gather(
                            out_ap=gt[:, 0:Cw, :],
                            in_ap=agouts[l][qq][:],
                            idxs_ap=idxt[:, w0:w0 + Cw * 8],
                            num_idxs=Cw * 128, num_idxs_reg=Cw * 128, elem_size=128,
                            single_packet=False, queue_num=_qrr[0] % 4)
                        _qrr[0] += 1
                        mtiles[(qq, g)] = gt
                    gd = s_ - (NQ - 1)
                    if 0 <= gd < NGRP:
                        for b in range(gd * K, min((gd + 1) * K, NBLK)):
                            emit_block(b)

            # ---------- head: partial logits ----------
            ph = psH.tile([128, POOLW], f32)
            nc.tensor.matmul(ph[0:1, :], lhsT=hw[:, 0:1], rhs=pooledT[:],
                             start=True, stop=True)
            outsb = cp.tile([1, POOLW], f32)
            nc.vector.tensor_copy(outsb[:], ph[0:1, :])
            nc.sync.dma_start(out_d[:], outsb[:])
    nc.compile()
    return nc


# ---------------------------------------------------------------------------
# PJRT compile-once runner (inlined; mirrors concourse.bass2jax.run_bass_via_pjrt)
# ---------------------------------------------------------------------------
class _Runner:
    def __init__(self, nc, n_cores):
        import jax
        import numpy as np
        from jax.sharding import Mesh, PartitionSpec
        from jax.experimental.shard_map import shard_map
        import concourse.mybir as mybir
        from concourse import bass2jax
        from concourse.bass2jax import _bass_exec_p, partition_id_tensor

        bass2jax.install_neuronx_cc_hook()
        self.jax = jax
        self.nc = nc
        self.n_cores = n_cores
        partition_name = nc.partition_id_tensor.name if nc.partition_id_tensor else None
        in_names, out_names, out_avals, zero_outs = [], [], [], []
        for alloc in nc.m.functions[0].allocations:
            if not isinstance(alloc, mybir.MemoryLocationSet):
                continue
            name = alloc.memorylocations[0].name
            if alloc.kind == "ExternalInput":
                if name != partition_name:
                    in_names.append(name)
            elif alloc.kind == "ExternalOutput":
                out_names.append(name)
                out_avals.append(jax.core.ShapedArray(tuple(alloc.tensor_shape),
                                                      mybir.dt.np(alloc.dtype)))
                zero_outs.append(np.zeros(tuple(alloc.tensor_shape),
                                          mybir.dt.np(alloc.dtype)))
        self.in_names, self.out_names = in_names, out_names
        self.out_avals, self.zero_outs = out_avals, zero_outs
        n_params, n_outs = len(in_names), len(out_avals)
        all_in = list(in_names) + list(out_names)
        if partition_name is not None:
            all_in.append(partition_name)

        def _body(*args):
            operands = list(args)
            if partition_name is not None:
                operands.append(partition_id_tensor())
            return tuple(_bass_exec_p.bind(
                *operands, out_avals=tuple(out_avals), in_names=tuple(all_in),
                out_names=tuple(out_names), lowering_input_output_aliases=(),
                sim_require_finite=False, sim_require_nnan=False, nc=nc))

        devices = jax.devices()[:n_cores]
        self.mesh = Mesh(np.asarray(devices), ("core",))
        in_specs = (PartitionSpec("core"),) * (n_params + n_outs)
        out_specs = (PartitionSpec("core"),) * n_outs
        self.sharded = jax.jit(
            shard_map(_body, mesh=self.mesh, in_specs=in_specs,
                      out_specs=out_specs, check_rep=False),
            donate_argnums=tuple(range(n_params, n_params + n_outs)),
            keep_unused=True)

    def run(self, in_maps):
        import numpy as np
        from jax.sharding import NamedSharding, PartitionSpec
        sharding = NamedSharding(self.mesh, PartitionSpec("core"))
        concat = [self.jax.device_put(
            np.concatenate([np.asarray(in_maps[c][n]) for c in range(self.n_cores)], axis=0),
            sharding) for n in self.in_names]
        zeros = [self.jax.device_put(
            np.zeros((self.n_cores * z.shape[0], *z.shape[1:]), z.dtype), sharding)
            for z in self.zero_outs]
        outs = self.sharded(*concat, *zeros)
        self.jax.block_until_ready(outs)
        return [
            {n: np.asarray(outs[i]).reshape(self.n_cores, *self.out_avals[i].shape)[c]
             for i, n in enumerate(self.out_names)}
            for c in range(self.n_cores)
        ]


_CACHE = {}


def kernel(x, edge_index, batch, Ws, bs, head_w, head_b):
    import hashlib
    ins_per_core, struct = _prep(x, edge_index, batch, Ws, bs, head_w, head_b)
    h = hashlib.sha1()
    h.update(np.ascontiguousarray(edge_index).tobytes())
    h.update(np.ascontiguousarray(batch).tobytes())
    h.update(str(struct["bs_nonzero"]).encode())
    key = h.hexdigest()
    if key not in _CACHE:
        nc = _build(struct)
        _CACHE[key] = _Runner(nc, NC)
        _CACHE["gcn"] = _CACHE[key]
    runner = _CACHE[key]
    results = runner.run(ins_per_core)
    out = np.zeros(G, np.float64)
    for c in range(NC):
        part = results[c]["out"].reshape(-1)
        g0 = int(struct["pooled_base"][c])
        w = min(POOLW, G - g0)
        out[g0:g0 + w] += part[:w]
    out += struct["head_b"]
    return out.astype(np.float32)
